# revision 14
# baseline (speedup 1.0000x reference)
"""Trainium2 Bass kernel: 3D 'same' convolution (implicit GEMM).

Problem: x (4, 64, 24, 24, 24) f32, weight (1, 128, 1728) f32
         -> out (4, 128, 24, 24, 24) f32  (SAME conv3d, k=3)

Sharding (8 cores): batch (4) x z-halves (2). Each core computes
out[b, :, z0:z0+12] for its (b, zh) shard; no inter-core communication.

Per-core algorithm: 27-tap implicit GEMM in bf16 (fp32 PSUM
accumulate; rel err ~2e-3 vs the fp32 reference). The PE array is
row-tiled 64x128: partitions 0-63 (tile_position (0,0)) and 64-127
((64,0)) hold identical copies of the zero-padded input window and
process disjoint halves of the 27 taps, accumulating into two separate
PSUM banks which are summed at evacuation (ACT copy + DVE add).
Alternating the two row halves hides each matmul's weight load and
drain under the other half's stream; the moving-operand feed sustains
~64 contraction rows/cycle, so 64-deep matmuls are the throughput
sweet spot (measured: 128-deep matmuls cost ~2x per column plus
unhidden per-matmul overhead, and deeper PSUM-bank interleaving only
adds overhead).

The padded input window (14 z-planes) is loaded as two overlapping
8-plane chunks so the second chunk's DMA hides under the first chunk's
matmuls. Output tiles are one z-plane x 21 y-rows x 24 (N=504, 2D
access pattern); the y=21..23 remainder rows are batched across 6
z-planes (N=432) per chunk.
"""

import sys

if "/opt/trn_rl_repo" not in sys.path:
    sys.path.insert(0, "/opt/trn_rl_repo")

import numpy as np

CIN, COUT, K = 64, 128, 3
DHW = 24  # cubic spatial extent
ZS = 12  # z-planes per shard
NP = 14  # padded z-planes per shard window (ZS + 2 halo)
PW = 26  # padded y/x extent
N_CORES = 8

# tap order: all 27 (dz, dy, dx)
TAPS = [(dz, dy, dx) for dz in range(3) for dy in range(3) for dx in range(3)]
N_T0 = 14  # taps on PE row-tile (0,0); the rest go to (64,0)


def _build_program(loop_n=None):
    """Build the SPMD Bass program (one NeuronCore's view).

    loop_n: if set, wrap the whole body in a hardware For_i loop with
    that many iterations (used by test.py for wall-clock timing).
    """
    import concourse.tile as tile
    from concourse import bacc, mybir

    F32 = mybir.dt.float32
    BF16 = mybir.dt.bfloat16

    t0_taps = TAPS[:N_T0]
    t8_taps = TAPS[N_T0:]

    nc = bacc.Bacc("TRN2")
    x_in = nc.declare_dram_parameter("x", [128, 3, NP, PW, 24], BF16, isOutput=False)
    wk_in = nc.declare_dram_parameter("wk", [128, N_T0, 128], BF16, isOutput=False)
    y_out = nc.declare_dram_parameter("y", [128, ZS, DHW, DHW], F32, isOutput=True)

    with tile.TileContext(nc) as tc:
        with (
            tc.tile_pool(name="xw", bufs=2) as xw_pool,
            tc.tile_pool(name="ps", bufs=3, space="PSUM") as ps_pool,
            tc.tile_pool(name="ob", bufs=3) as ob_pool,
        ):

            def body(_iv=None):
                W = xw_pool.tile([128, N_T0, 128], BF16, name="W")
                nc.sync.dma_start(out=W[:], in_=wk_in[:])
                XA = xw_pool.tile([128, 3, 8, PW, 24], BF16, name="XA")
                XB = xw_pool.tile([128, 3, 8, PW, 24], BF16, name="XB")
                nc.sync.dma_start(out=XA[:], in_=x_in[:, :, 0:8])
                nc.sync.dma_start(out=XB[:], in_=x_in[:, :, 6:14])

                # output tiles: ("plane", chunk, zoff, z, r0, nr) covering
                # y rows [r0, r0+nr) of a z-plane (N=nr*24), split 11+10 so
                # shorter matmul chains expose more fill/drain overlap; or
                # ("rem", chunk, zoff, None, 0, 0) N=432 (6x3x24, 3D)
                tiles = (
                    [("plane", XA, 0, z, r0, nr) for z in range(6) for r0, nr in ((0, 11), (11, 10))]
                    + [("rem", XA, 0, None, 0, 0)]
                    + [("plane", XB, 6, z, r0, nr) for z in range(6, 12) for r0, nr in ((0, 11), (11, 10))]
                    + [("rem", XB, 6, None, 0, 0)]
                )

                def rhs_ap(X, zoff, kind, z, r0, nr, dz, dy, dx, lo, hi):
                    # dx is baked into the pre-shifted variant (index 1), so
                    # plane reads are one contiguous nr*24-element run per
                    # partition and rem reads are 6 contiguous 72-runs
                    if kind == "plane":
                        return X[lo:hi, dx, z - zoff + dz, dy + r0 : dy + r0 + nr, 0:24]
                    return X[lo:hi, dx, dz : dz + 6, 21 + dy : 24 + dy, 0:24]

                for kind, X, zoff, z, r0, nr in tiles:
                    n = nr * 24 if kind == "plane" else 432
                    ps0 = ps_pool.tile([128, 512], F32, name="ps0", tag="ps0")
                    ps1 = ps_pool.tile([128, 512], F32, name="ps1", tag="ps1")
                    n0, n1 = len(t0_taps), len(t8_taps)
                    for i in range(n0):
                        dz, dy, dx = t0_taps[i]
                        nc.tensor.matmul(
                            ps0[:, :n],
                            lhsT=W[0:64, i, :],
                            rhs=rhs_ap(X, zoff, kind, z, r0, nr, dz, dy, dx, 0, 64),
                            start=(i == 0),
                            stop=(i == n0 - 1),
                            skip_group_check=True,
                            tile_position=(0, 0),
                        )
                        if i < n1:
                            dz, dy, dx = t8_taps[i]
                            nc.tensor.matmul(
                                ps1[:, :n],
                                lhsT=W[64:128, i, :],
                                rhs=rhs_ap(X, zoff, kind, z, r0, nr, dz, dy, dx, 64, 128),
                                start=(i == 0),
                                stop=(i == n1 - 1),
                                skip_group_check=True,
                                tile_position=(64, 0),
                            )
                    tmp = ob_pool.tile([128, 512], F32, name="tmp", tag="tmp")
                    nc.scalar.copy(tmp[:, :n], ps1[:, :n])
                    ob = ob_pool.tile([128, 512], F32, name="ob", tag="ob")
                    nc.vector.tensor_add(ob[:, :n], ps0[:, :n], tmp[:, :n])
                    if kind == "plane":
                        nc.sync.dma_start(
                            out=y_out[:, z, r0 : r0 + nr, :], in_=ob[:, :n]
                        )
                    else:
                        # one DMA per z-plane: keeps each transfer one
                        # contiguous run per partition (descriptor-lean)
                        for j in range(6):
                            nc.sync.dma_start(
                                out=y_out[:, zoff + j, 21:24, :],
                                in_=ob[:, j * 72 : (j + 1) * 72],
                            )

            if loop_n is not None:
                # 2x unroll: alternate xw_pool buffers across the two body
                # copies so the next iteration's weight/input DMAs land in
                # the idle buffer set while the current one computes
                # (single-buffered hw loops serialize the W reload against
                # the last matmul of the previous iteration).
                assert loop_n % 2 == 0, loop_n
                with tc.For_i(0, loop_n // 2, 1) as _i:
                    body(_i)
                    body(_i)
            else:
                body()

    nc.finalize()
    return nc


def _make_in_maps(x, weight):
    import ml_dtypes

    BF16 = ml_dtypes.bfloat16
    w = np.asarray(weight, np.float32).reshape(COUT, CIN, K, K, K)
    wk = np.zeros((128, N_T0, 128), BF16)
    for i, (dz, dy, dx) in enumerate(TAPS[:N_T0]):
        wk[0:64, i, :] = w[:, :, dz, dy, dx].T.astype(BF16)
    for i, (dz, dy, dx) in enumerate(TAPS[N_T0:]):
        wk[64:128, i, :] = w[:, :, dz, dy, dx].T.astype(BF16)

    in_maps = []
    for c in range(N_CORES):
        b, zh = divmod(c, 2)
        z0 = zh * ZS
        xpad = np.zeros((CIN, PW, PW, PW), BF16)
        xpad[:, 1:25, 1:25, 1:25] = x[b].astype(BF16)
        win = xpad[:, z0 : z0 + NP]  # (64, 14, 26, 26)
        # three x-shifted variants, each x-contiguous 24 wide: variant v
        # holds win[..., v:v+24] so tap dx=v reads x offsets [0, 24)
        V = np.stack([win[:, :, :, v : v + 24] for v in range(3)], axis=1)
        X = np.empty((128, 3, NP, PW, 24), BF16)
        X[0:64] = V
        X[64:128] = V
        in_maps.append({"x": X, "wk": wk})
    return in_maps


def _gather(results):
    out = np.empty((4, COUT, DHW, DHW, DHW), np.float32)
    for c in range(N_CORES):
        b, zh = divmod(c, 2)
        out[b, :, zh * ZS : (zh + 1) * ZS] = results[c]["y"]
    return out


def kernel(x, weight):
    from concourse.bass_utils import run_bass_kernel_spmd

    x = np.asarray(x, np.float32)
    in_maps = _make_in_maps(x, weight)
    nc = _build_program()
    res = run_bass_kernel_spmd(nc, in_maps, list(range(N_CORES)))
    return _gather(res.results)



# revision 16
# speedup vs baseline: 1.1296x; 1.1296x over previous
"""Trainium2 Bass kernel: 3D 'same' convolution (implicit GEMM).

Problem: x (4, 64, 24, 24, 24) f32, weight (1, 128, 1728) f32
         -> out (4, 128, 24, 24, 24) f32  (SAME conv3d, k=3)

Sharding (8 cores): batch (4) x z-halves (2). Each core computes
out[b, :, z0:z0+12] for its (b, zh) shard; no inter-core communication.

Per-core algorithm: 27-tap implicit GEMM in bf16 (fp32 PSUM
accumulate; rel err ~2e-3 vs the fp32 reference). The PE array is
row-tiled 64x128: partitions 0-63 (tile_position (0,0)) and 64-127
((64,0)) hold identical copies of the zero-padded input window and
process disjoint halves of the 27 taps, accumulating into two separate
PSUM banks which are summed at evacuation (ACT copy + DVE add).
Alternating the two row halves hides each matmul's weight load and
drain under the other half's stream. Two measured laws shape the
tiling: (1) short accumulation chains overlap across the row halves
where long chains serialize, and (2) that overlap only materializes
when each matmul's moving operand is one contiguous run per
partition. So x is staged as three dx-pre-shifted variants in
x-contiguous 24-wide layout, and each z-plane's output is computed as
two 12-row tiles (N=288) whose rhs reads are single contiguous runs
-- no strided windows, no remainder tiles.

The padded input window (14 z-planes) is loaded as two overlapping
8-plane chunks so the second chunk's DMA hides under the first
chunk's matmuls; the timing loop is 2x-unrolled with alternating
input buffers so each iteration's DMAs land during the previous
iteration's compute.
"""

import sys

if "/opt/trn_rl_repo" not in sys.path:
    sys.path.insert(0, "/opt/trn_rl_repo")

import numpy as np

CIN, COUT, K = 64, 128, 3
DHW = 24  # cubic spatial extent
ZS = 12  # z-planes per shard
NP = 14  # padded z-planes per shard window (ZS + 2 halo)
PW = 26  # padded y/x extent
N_CORES = 8

# tap order: all 27 (dz, dy, dx)
TAPS = [(dz, dy, dx) for dz in range(3) for dy in range(3) for dx in range(3)]
N_T0 = 14  # taps on PE row-tile (0,0); the rest go to (64,0)


def _build_program(loop_n=None):
    """Build the SPMD Bass program (one NeuronCore's view).

    loop_n: if set, wrap the whole body in a hardware For_i loop with
    that many iterations (used by test.py for wall-clock timing).
    """
    import concourse.tile as tile
    from concourse import bacc, mybir

    F32 = mybir.dt.float32
    BF16 = mybir.dt.bfloat16

    t0_taps = TAPS[:N_T0]
    t8_taps = TAPS[N_T0:]

    nc = bacc.Bacc("TRN2")
    x_in = nc.declare_dram_parameter("x", [128, 3, NP, PW, 24], BF16, isOutput=False)
    wk_in = nc.declare_dram_parameter("wk", [128, N_T0, 128], BF16, isOutput=False)
    y_out = nc.declare_dram_parameter("y", [128, ZS, DHW, DHW], F32, isOutput=True)

    with tile.TileContext(nc) as tc:
        with (
            tc.tile_pool(name="xw", bufs=2) as xw_pool,
            tc.tile_pool(name="ps", bufs=3, space="PSUM") as ps_pool,
            tc.tile_pool(name="ob", bufs=3) as ob_pool,
        ):

            def body(_iv=None):
                W = xw_pool.tile([128, N_T0, 128], BF16, name="W")
                nc.sync.dma_start(out=W[:], in_=wk_in[:])
                XA = xw_pool.tile([128, 3, 8, PW, 24], BF16, name="XA")
                XB = xw_pool.tile([128, 3, 8, PW, 24], BF16, name="XB")
                nc.sync.dma_start(out=XA[:], in_=x_in[:, :, 0:8])
                nc.sync.dma_start(out=XB[:], in_=x_in[:, :, 6:14])

                # output tiles: two 12-row halves per z-plane (N=288).
                # The y window has 26 rows (2 pad), so dy + r0 + 12 <= 26
                # stays in range for both halves and no remainder tile is
                # needed; every rhs read is one contiguous 288-element run
                # per partition (dx baked into the pre-shifted variant).
                tiles = (
                    [("plane", XA, 0, z, r0, 12) for z in range(6) for r0 in (0, 12)]
                    + [("plane", XB, 6, z, r0, 12) for z in range(6, 12) for r0 in (0, 12)]
                )

                def rhs_ap(X, zoff, kind, z, r0, nr, dz, dy, dx, lo, hi):
                    return X[lo:hi, dx, z - zoff + dz, dy + r0 : dy + r0 + nr, 0:24]

                for kind, X, zoff, z, r0, nr in tiles:
                    n = nr * 24
                    ps0 = ps_pool.tile([128, 512], F32, name="ps0", tag="ps0")
                    ps1 = ps_pool.tile([128, 512], F32, name="ps1", tag="ps1")
                    n0, n1 = len(t0_taps), len(t8_taps)
                    for i in range(n0):
                        dz, dy, dx = t0_taps[i]
                        nc.tensor.matmul(
                            ps0[:, :n],
                            lhsT=W[0:64, i, :],
                            rhs=rhs_ap(X, zoff, kind, z, r0, nr, dz, dy, dx, 0, 64),
                            start=(i == 0),
                            stop=(i == n0 - 1),
                            skip_group_check=True,
                            tile_position=(0, 0),
                        )
                        if i < n1:
                            dz, dy, dx = t8_taps[i]
                            nc.tensor.matmul(
                                ps1[:, :n],
                                lhsT=W[64:128, i, :],
                                rhs=rhs_ap(X, zoff, kind, z, r0, nr, dz, dy, dx, 64, 128),
                                start=(i == 0),
                                stop=(i == n1 - 1),
                                skip_group_check=True,
                                tile_position=(64, 0),
                            )
                    tmp = ob_pool.tile([128, 512], F32, name="tmp", tag="tmp")
                    nc.scalar.copy(tmp[:, :n], ps1[:, :n])
                    ob = ob_pool.tile([128, 512], F32, name="ob", tag="ob")
                    nc.vector.tensor_add(ob[:, :n], ps0[:, :n], tmp[:, :n])
                    nc.sync.dma_start(
                        out=y_out[:, z, r0 : r0 + nr, :], in_=ob[:, :n]
                    )

            if loop_n is not None:
                # 2x unroll: alternate xw_pool buffers across the two body
                # copies so the next iteration's weight/input DMAs land in
                # the idle buffer set while the current one computes
                # (single-buffered hw loops serialize the W reload against
                # the last matmul of the previous iteration).
                assert loop_n % 2 == 0, loop_n
                with tc.For_i(0, loop_n // 2, 1) as _i:
                    body(_i)
                    body(_i)
            else:
                body()

    nc.finalize()
    return nc


def _make_in_maps(x, weight):
    import ml_dtypes

    BF16 = ml_dtypes.bfloat16
    w = np.asarray(weight, np.float32).reshape(COUT, CIN, K, K, K)
    wk = np.zeros((128, N_T0, 128), BF16)
    for i, (dz, dy, dx) in enumerate(TAPS[:N_T0]):
        wk[0:64, i, :] = w[:, :, dz, dy, dx].T.astype(BF16)
    for i, (dz, dy, dx) in enumerate(TAPS[N_T0:]):
        wk[64:128, i, :] = w[:, :, dz, dy, dx].T.astype(BF16)

    in_maps = []
    for c in range(N_CORES):
        b, zh = divmod(c, 2)
        z0 = zh * ZS
        xpad = np.zeros((CIN, PW, PW, PW), BF16)
        xpad[:, 1:25, 1:25, 1:25] = x[b].astype(BF16)
        win = xpad[:, z0 : z0 + NP]  # (64, 14, 26, 26)
        # three x-shifted variants, each x-contiguous 24 wide: variant v
        # holds win[..., v:v+24] so tap dx=v reads x offsets [0, 24)
        V = np.stack([win[:, :, :, v : v + 24] for v in range(3)], axis=1)
        X = np.empty((128, 3, NP, PW, 24), BF16)
        X[0:64] = V
        X[64:128] = V
        in_maps.append({"x": X, "wk": wk})
    return in_maps


def _gather(results):
    out = np.empty((4, COUT, DHW, DHW, DHW), np.float32)
    for c in range(N_CORES):
        b, zh = divmod(c, 2)
        out[b, :, zh * ZS : (zh + 1) * ZS] = results[c]["y"]
    return out


def kernel(x, weight):
    from concourse.bass_utils import run_bass_kernel_spmd

    x = np.asarray(x, np.float32)
    in_maps = _make_in_maps(x, weight)
    nc = _build_program()
    res = run_bass_kernel_spmd(nc, in_maps, list(range(N_CORES)))
    return _gather(res.results)

